# revision 1
# baseline (speedup 1.0000x reference)
"""Trainium2 Bass kernel for nn_RelationDecoder (ragged_sequence).

Strategy (8 NeuronCores, SPMD, no collectives):
  - Shard the exp-entity axis k (128 -> 8 x 16).  Every einsum stage
    (hep, tep, tcls, pred) is blockwise in k, so all heavy work shards;
    only the cheap LSTM position tables (~2.4 GF) replicate.
  - LSTM pooling: the span gathers index only 512 distinct positions, so
    compute h = LSTMCell(enc[t]) for all 512 positions (one matmul chain
    per pool/direction), store h-tables in DRAM, then indirect-DMA gather
    the spans and max-reduce.
  - hep/tep:  U[k,a,b] = sum_c T[a,b,c] * Ee[k,c]   (T streamed once as the
    moving operand, host-pretransposed to [c,a,b]; Ee^T stationary with
    4x PE column tiling so PSUM uses all 128 partitions), round-trip U
    through DRAM to flip the partition axis, then
    hepT[k][b,i] = sum_a U[k,a,b] * He[i,a].
  - pred: tclsT[k][c,(j)] per m = sum_b T_cls[b,m,c] tep[k][j,b]; then
    pred[k][i,m,j] = sum_c hepT[k][c,i] * tclsT[k][c,j] accumulated over c.
  - All matmuls in bf16 (fp32 PSUM accumulate).  Measured end-to-end
    absmax-relative error of this quantization vs the fp32 reference:
    ~8e-3.

Host side: casts/transposes inputs (bf16, [c,a,b] layouts, gate slimming:
the f-gate is dead because h0=c0=0), runs the same NEFF on cores 0-7 with
per-core exp_idx shards, reassembles pred on the host.
"""

import os
import tempfile

import numpy as np
import ml_dtypes

import concourse.bass as bass
import concourse.mybir as mybir
import concourse.tile as tile
from concourse import bacc
from concourse.bass import IndirectOffsetOnAxis
from concourse.masks import make_identity

SEQ, D, HID, C = 512, 768, 128, 5
NENT, SPAN = 128, 16
H2 = 2 * HID            # 256
NCORES = 8
KSH = NENT // NCORES    # 16 exp entities per core
P = 128
DAUG = D + P            # 768 data rows + [ones row + zero pad] = 896
NDCH = DAUG // P        # 7 contraction chunks for the gate matmul
GATES = 3 * HID         # 384: i, g, o gates (f gate is dead)
ATILE = 8               # a-rows per stage-A DMA tile (4 col-group MMs)

BF = mybir.dt.bfloat16
F32 = mybir.dt.float32
I32 = mybir.dt.int32
AF = mybir.ActivationFunctionType
ALU = mybir.AluOpType

_NC_CACHE = {}


class _CopyVia:
    def __init__(self, eng, is_vec):
        self.eng, self.is_vec = eng, is_vec

    def tensor_copy(self, out, in_):
        if self.is_vec:
            self.eng.tensor_copy(out=out, in_=in_)
        else:
            self.eng.copy(out, in_)


def _copy_engine(nc, idx):
    """Alternate PSUM->SBUF copies between DVE and ACT to split the load."""
    return _CopyVia(nc.vector, True) if idx % 2 == 0 else _CopyVia(nc.scalar, False)


def build_nc():
    """Build + bacc-compile the Bass module (cached)."""
    if "nc" in _NC_CACHE:
        return _NC_CACHE["nc"]

    nc = bacc.Bacc(
        "TRN2",
        target_bir_lowering=False,
        debug=False,
        enable_asserts=False,
        num_devices=NCORES,
    )

    # ---- I/O -------------------------------------------------------------
    enc_t = nc.dram_tensor("enc_t", [DAUG, SEQ], BF, kind="ExternalInput")
    wts = nc.dram_tensor("wts", [6, DAUG, GATES], BF, kind="ExternalInput")
    t_he = nc.dram_tensor("t_he", [H2, H2, H2], BF, kind="ExternalInput")  # [c,a,b]
    t_te = nc.dram_tensor("t_te", [H2, H2, H2], BF, kind="ExternalInput")  # [c,a,b]
    t_cls = nc.dram_tensor("t_cls", [H2, C, H2], BF, kind="ExternalInput")  # [b,m,c]
    hold_idx = nc.dram_tensor("hold_idx", [NENT, SPAN], I32, kind="ExternalInput")
    targ_idx = nc.dram_tensor("targ_idx", [NENT, SPAN], I32, kind="ExternalInput")
    exp_idx = nc.dram_tensor("exp_idx", [KSH, SPAN], I32, kind="ExternalInput")
    # per-core output, layout [k, i, m, j]; host reorders to [i, j, k, m]
    pred_out = nc.dram_tensor("pred_out", [KSH, NENT, C, NENT], F32,
                              kind="ExternalOutput")

    # h tables must be dedicated DRAM tensors: indirect-DMA gather sources
    # need AP offset 0.
    h_tab = [nc.dram_tensor(f"h_tab_{p}", [SEQ, H2], BF, kind="Internal")
             for p in range(3)]

    from contextlib import ExitStack
    with tile.TileContext(nc) as tcx, ExitStack() as stk:
        const = stk.enter_context(tcx.tile_pool(name="const", bufs=1))
        work = stk.enter_context(tcx.tile_pool(name="work", bufs=3))
        dram = stk.enter_context(tcx.tile_pool(name="dram", bufs=1, space="DRAM"))
        gpool = stk.enter_context(tcx.tile_pool(name="gpool", bufs=2))

        # ---- persistent SBUF loads --------------------------------------
        enc_sb = const.tile([P, NDCH, SEQ], BF)
        nc.sync.dma_start(enc_sb[:], enc_t.ap().rearrange("(n p) s -> p n s", p=P))
        wt_sb = const.tile([P, NDCH, 6, GATES], BF)
        for w in range(6):
            nc.sync.dma_start(wt_sb[:, :, w, :],
                              wts.ap()[w].rearrange("(n p) g -> p n g", p=P))
        tcls_sb = const.tile([P, 2, C, H2], BF)
        nc.sync.dma_start(tcls_sb[:], t_cls.ap().rearrange("(bc p) m c -> p bc m c", p=P))
        hold_sb = const.tile([NENT, SPAN], I32)
        nc.sync.dma_start(hold_sb[:], hold_idx.ap())
        targ_sb = const.tile([NENT, SPAN], I32)
        nc.sync.dma_start(targ_sb[:], targ_idx.ap())
        exp_sb = const.tile([KSH, SPAN], I32)
        nc.sync.dma_start(exp_sb[:], exp_idx.ap())
        ident = const.tile([P, P], BF)
        make_identity(nc, ident[:])

        # ---- phase 1: LSTM h tables at all 512 positions ----------------
        # G[t, g] = enc_aug[t, :] @ WT_aug[:, g]  (bias folded in via the
        # ones row of enc_aug); h = sig(o) * tanh(sig(i) * tanh(g)).
        with tcx.tile_pool(name="ps1", bufs=2, space="PSUM") as ps1:
            for p in range(3):
                for tb in range(SEQ // P):
                    h_tile = work.tile([P, H2], BF, tag="h_tile")
                    for d in range(2):
                        ps_g = ps1.tile([P, GATES], F32, tag="ps_g")
                        for dc in range(NDCH):
                            nc.tensor.matmul(
                                ps_g[:],
                                lhsT=enc_sb[:, dc, tb * P:(tb + 1) * P],
                                rhs=wt_sb[:, dc, 2 * p + d, :],
                                start=(dc == 0),
                                stop=(dc == NDCH - 1),
                            )
                        ti = work.tile([P, HID], F32, tag="ti")
                        nc.scalar.activation(ti[:], ps_g[:, 0:HID], AF.Sigmoid)
                        tg = work.tile([P, HID], F32, tag="tg")
                        nc.scalar.activation(tg[:], ps_g[:, HID:2 * HID], AF.Tanh)
                        cc_ = work.tile([P, HID], F32, tag="cc_")
                        nc.vector.tensor_mul(cc_[:], ti[:], tg[:])
                        tc_ = work.tile([P, HID], F32, tag="tc_")
                        nc.scalar.activation(tc_[:], cc_[:], AF.Tanh)
                        to = work.tile([P, HID], F32, tag="to")
                        nc.scalar.activation(to[:], ps_g[:, 2 * HID:3 * HID], AF.Sigmoid)
                        nc.vector.tensor_mul(h_tile[:, d * HID:(d + 1) * HID],
                                             to[:], tc_[:])
                    nc.sync.dma_start(h_tab[p][tb * P:(tb + 1) * P, :], h_tile[:])

        # ---- phase 2: span gather + maxpool + transposes -----------------
        def gather_pool(pool_i, idx_sb, nent, tag):
            g_t = gpool.tile([NENT, SPAN, H2], BF, tag="gath", name=f"g_{tag}")[:nent]
            for l in range(SPAN):
                nc.gpsimd.indirect_dma_start(
                    out=g_t[:, l, :],
                    out_offset=None,
                    in_=h_tab[pool_i].ap(),
                    in_offset=IndirectOffsetOnAxis(ap=idx_sb[:, l:l + 1], axis=0),
                )
            pooled = const.tile([nent, H2], BF, tag=f"pool_{tag}")
            nc.vector.reduce_max(
                out=pooled[:],
                in_=g_t[:].rearrange("p l f -> p f l"),
                axis=mybir.AxisListType.X,
            )
            return pooled

        he_sb = gather_pool(0, hold_sb, NENT, "he")
        te_sb = gather_pool(1, targ_sb, NENT, "te")
        ee_sb = gather_pool(2, exp_sb, KSH, "ee")

        with tcx.tile_pool(name="ps2", bufs=2, space="PSUM") as ps2:
            def transpose_to(src, nrows, ncols, tag):
                # src [nrows, H2] -> dst [P, 2, ncols>=nrows], zero-padded cols
                # (the zero pad lets stage-A col-tiled matmuls write all 128
                # PSUM partitions so full-width PSUM->SBUF copies are legal).
                dst = const.tile([P, 2, ncols], BF, tag=f"T_{tag}")
                if ncols > nrows:
                    nc.any.memzero(dst[:])
                for ch in range(2):
                    ps_t = ps2.tile([P, P], BF, tag="ps_tr")
                    nc.tensor.transpose(
                        ps_t[:, :nrows],
                        src[:, ch * P:(ch + 1) * P],
                        ident[:nrows, :nrows],
                    )
                    _copy_engine(nc, ch).tensor_copy(out=dst[:, ch, :nrows],
                                                     in_=ps_t[:, :nrows])
                return dst

            heT = transpose_to(he_sb, NENT, NENT, "he")
            teT = transpose_to(te_sb, NENT, NENT, "te")
            eeT = transpose_to(ee_sb, KSH, 32, "ee")

        # ---- phase 3+4 per T tensor: U = T x Ee, then eT = U x He/Te -----
        u_dram = {n: dram.tile([KSH, H2, H2], BF, tag=f"u_{n}", name=f"u_{n}")
                  for n in ("te", "he")}
        epT = {n: const.tile([P, 2, KSH, P], BF, tag=f"{n}pT", name=f"{n}pT")
               for n in ("te", "he")}

        ci = 0
        # T_te first: its downstream (tclsT) is the longest chain.
        for name, t_dram, sT in (("te", t_te, teT), ("he", t_he, heT)):
            ud = u_dram[name]
            # stage A: U[k, a, b] = sum_c T[c, a, b] * Ee[k, c]
            with tcx.tile_pool(name=f"ps3{name}", bufs=3, space="PSUM") as ps3:
                for ab in range(H2 // ATILE):
                    rhs = []
                    for cc in range(2):
                        r = work.tile([P, ATILE, H2], BF, tag=f"stA{cc}")
                        nc.sync.dma_start(
                            r[:],
                            t_dram.ap()[cc * P:(cc + 1) * P,
                                        ab * ATILE:(ab + 1) * ATILE, :],
                        )
                        rhs.append(r)
                    ps_u = ps3.tile([P, 512], F32, tag="ps_u")
                    for g in range(4):
                        for cc in range(2):
                            nc.tensor.matmul(
                                ps_u[32 * g:32 * g + 32, :],
                                lhsT=eeT[:, cc, :],
                                rhs=rhs[cc][:, 2 * g:2 * g + 2, :],
                                start=(cc == 0),
                                stop=(cc == 1),
                                tile_position=(0, 32 * g),
                            )
                    u_sb = work.tile([P, 512], BF, tag="u_sb")
                    _copy_engine(nc, ci).tensor_copy(out=u_sb[:], in_=ps_u[:])
                    ci += 1
                    for g in range(4):
                        a0 = ab * ATILE + 2 * g
                        nc.sync.dma_start(ud[:, a0:a0 + 2, :],
                                          u_sb[32 * g:32 * g + KSH, :])

            # stage B: epT[k][b, i] = sum_a U[k, a, b] * sT[a, i]
            with tcx.tile_pool(name=f"ps4{name}", bufs=2, space="PSUM") as ps4:
                for k in range(KSH):
                    uk = work.tile([P, 2, H2], BF, tag="uk")
                    nc.sync.dma_start(
                        uk[:], ud[k].rearrange("(ac p) b -> p ac b", p=P))
                    ps_h = ps4.tile([P, H2], F32, tag="ps_h")
                    for bc in range(2):
                        for ac in range(2):
                            nc.tensor.matmul(
                                ps_h[:, bc * P:(bc + 1) * P],
                                lhsT=uk[:, ac, bc * P:(bc + 1) * P],
                                rhs=sT[:, ac, :],
                                start=(ac == 0),
                                stop=(ac == 1),
                            )
                    _copy_engine(nc, k).tensor_copy(
                        out=epT[name][:, :, k, :],
                        in_=ps_h[:].rearrange("p (bc i) -> p bc i", bc=2))

        tepT, hepT = epT["te"], epT["he"]

        # ---- phase 5: tclsT[k][c, j] per m = sum_b Tcls[b, m, c] tep[k][j, b]
        tclsT = const.tile([P, 2, C, KSH, P], BF, tag="tclsT")
        with tcx.tile_pool(name="ps5", bufs=2, space="PSUM") as ps5:
            ci5 = 0
            for m in range(C):
                for cc in range(2):
                    # one MM spans 4 k-slices (N=512 = one PSUM bank), so each
                    # bank holds exactly one open accumulation group over bc.
                    ps_t5 = ps5.tile([P, 4, 512], F32, tag="ps_t5")
                    for bc in range(2):
                        for q in range(4):
                            nc.tensor.matmul(
                                ps_t5[:, q, :],
                                lhsT=tcls_sb[:, bc, m, cc * P:(cc + 1) * P],
                                rhs=tepT[:, bc, 4 * q:4 * (q + 1), :],
                                start=(bc == 0),
                                stop=(bc == 1),
                            )
                    _copy_engine(nc, ci5).tensor_copy(
                        out=tclsT[:, cc, m, :, :],
                        in_=ps_t5[:].rearrange("p q (kk j) -> p (q kk) j", j=P))
                    ci5 += 1

        # ---- phase 6: pred[k][i, m, j] = sum_cc hepT[k].T @ tclsT[k] ------
        with tcx.tile_pool(name="ps6", bufs=2, space="PSUM") as ps6:
            for k in range(KSH):
                ps_p = ps6.tile([P, C, P], F32, tag="ps_p")
                for cc in range(2):
                    # m 0..3 in bank 0 (one N=512 MM), m=4 in bank 1: one open
                    # accumulation group per bank.
                    nc.tensor.matmul(
                        ps_p[:, 0:4, :],
                        lhsT=hepT[:, cc, k, :],
                        rhs=tclsT[:, cc, 0:4, k, :],
                        start=(cc == 0),
                        stop=(cc == 1),
                    )
                    nc.tensor.matmul(
                        ps_p[:, 4, :],
                        lhsT=hepT[:, cc, k, :],
                        rhs=tclsT[:, cc, 4, k, :],
                        start=(cc == 0),
                        stop=(cc == 1),
                    )
                pred_sb = work.tile([P, C, P], F32, tag="pred_sb")
                _copy_engine(nc, k).tensor_copy(out=pred_sb[:], in_=ps_p[:])
                nc.sync.dma_start(pred_out.ap()[k], pred_sb[:])

    nc.compile()
    _NC_CACHE["nc"] = nc
    return nc


def prep_inputs(inputs):
    """Host-side packing: bf16 casts, transposed layouts, gate slimming."""
    bf16 = ml_dtypes.bfloat16
    enc = np.asarray(inputs["encoder_output"], np.float32)[0]        # [SEQ, D]
    enc_aug = np.zeros((DAUG, SEQ), np.float32)
    enc_aug[:D] = enc.T
    enc_aug[D] = 1.0                                                 # bias row

    wts = np.zeros((6, DAUG, GATES), np.float32)
    for wi, (wn, bn) in enumerate([("Wh_f", "bh_f"), ("Wh_b", "bh_b"),
                                   ("Wt_f", "bt_f"), ("Wt_b", "bt_b"),
                                   ("We_f", "be_f"), ("We_b", "be_b")]):
        W = np.asarray(inputs[wn], np.float32)                       # [4H, D]
        b = np.asarray(inputs[bn], np.float32)                       # [4H]
        # gate order i,f,g,o; keep i,g,o (f is dead since c0=0)
        keep = np.r_[0:HID, 2 * HID:4 * HID]
        wts[wi, :D] = W[keep].T                                      # [D, 384]
        wts[wi, D] = b[keep]

    shared = {
        "enc_t": enc_aug.astype(bf16),
        "wts": wts.astype(bf16),
        "t_he": np.ascontiguousarray(
            np.asarray(inputs["T_he"], np.float32).transpose(2, 0, 1)).astype(bf16),
        "t_te": np.ascontiguousarray(
            np.asarray(inputs["T_te"], np.float32).transpose(2, 0, 1)).astype(bf16),
        "t_cls": np.ascontiguousarray(
            np.asarray(inputs["T_cls"], np.float32)).astype(bf16),
        "hold_idx": np.ascontiguousarray(np.asarray(inputs["holder_idxs"], np.int32)),
        "targ_idx": np.ascontiguousarray(np.asarray(inputs["target_idxs"], np.int32)),
    }
    exp = np.asarray(inputs["exp_idxs"], np.int32)
    in_maps = []
    for c in range(NCORES):
        m = dict(shared)
        m["exp_idx"] = np.ascontiguousarray(exp[c * KSH:(c + 1) * KSH])
        in_maps.append(m)
    return in_maps


def kernel(**inputs) -> np.ndarray:
    nc = build_nc()
    in_maps = prep_inputs(inputs)
    trace = bool(int(os.environ.get("KERNEL_TRACE", "0")))
    kwargs = {}
    if trace:
        kwargs = dict(trace=True, tmpdir=tempfile.mkdtemp(prefix="rd_neff_"))
    res = bass_run(nc, in_maps, **kwargs)
    outs = [r["pred_out"] for r in res.results]        # [k, i, m, j] each
    full = np.concatenate([o.transpose(1, 3, 0, 2) for o in outs], axis=2)
    if trace:
        kernel.last_result = res
    return np.ascontiguousarray(full)                   # [i, j, k, m] fp32


def bass_run(nc, in_maps, **kwargs):
    from concourse.bass_utils import run_bass_kernel_spmd
    return run_bass_kernel_spmd(nc, in_maps, core_ids=list(range(NCORES)), **kwargs)


if __name__ == "__main__":
    import reference
    inputs = reference.setup_inputs()
    out = kernel(**{k: np.asarray(v) for k, v in inputs.items()})
    print("kernel output", out.shape, out.dtype)

